# revision 4
# baseline (speedup 1.0000x reference)
"""MinkowskiEngine transposed-conv + ReLU Bass kernel for 8 TRN2 cores. v7 (fp16, NACC=4, 32K rings, 1K sweep)

Strategy (output-partitioned, no collectives):
  - core c owns output rows [c*RPC, (c+1)*RPC)
  - feats stored bf16; dma_gather(transpose=True) delivers featsT chunks
    directly (no PE transposes); windows of 2048 pairs = 16 k-pure,
    block-pure 128-pair subchunks
  - GEMM per subchunk: 2 bf16 matmuls (c_in halves) accumulate in PSUM
  - contrib copied PSUM->SBUF fp32 by ACT/DVE alternating
  - dma_scatter_add (fp32) into one of NACC DRAM accumulators; host
    guarantees distinct rows within each 2048-pair window
  - final sweep: 512-row tiles, acc0(+bias-seeded) + acc1, ReLU, store
"""
import numpy as np
from contextlib import ExitStack

import concourse.bass as bass
import concourse.bacc as bacc
from concourse import mybir

P = 128
SUB_PER_WIN = 4
WIN_PAIRS = P * SUB_PER_WIN  # 2048
NACC = 4
NQ = 4
SWT = 1024                   # sweep tile rows


# ---------------------------------------------------------------------------
# host-side schedule construction (same structure as baseline, coarser windows)
# ---------------------------------------------------------------------------

def build_schedule(in_map, out_map, n_in, n_out, n_cores, k_off, rng_seed=0):
    K, M = in_map.shape
    rpc = -(-n_out // n_cores)
    rpc = -(-rpc // SWT) * SWT          # pad to sweep tile
    block_rows = -(-n_in // 4)
    block_rows = -(-block_rows // P) * P
    n_blocks = -(-n_in // block_rows)
    assert block_rows <= 32767 and rpc + 256 <= 65536
    acc_rows = rpc + 256
    trash = rpc

    kk, ii = np.meshgrid(np.arange(K, dtype=np.int64), np.arange(M, dtype=np.int64),
                         indexing="ij")
    irow = in_map.astype(np.int64).ravel()
    orow = out_map.astype(np.int64).ravel()
    kfl = kk.ravel()
    core = orow // rpc
    blk = irow // block_rows
    irel = irow - blk * block_rows
    orel = orow - core * rpc

    S = np.zeros((n_blocks, K), np.int64)
    per_core_pairs = []
    for c in range(n_cores):
        m = core == c
        per_core_pairs.append((kfl[m], blk[m], irel[m], orel[m]))
        cnt = np.zeros((n_blocks, K), np.int64)
        np.add.at(cnt, (blk[m], kfl[m]), 1)
        S = np.maximum(S, -(-cnt // P))
    S = np.maximum(S, 1)
    # capacity for row-uniqueness: a row with m occurrences of (b,k) needs
    # m subchunks of (b,k) in distinct windows
    for c in range(n_cores):
        kc, bc, ic, oc = per_core_pairs[c]
        key = (bc * K + kc) * (np.int64(1) << 32) + oc
        _, mult = np.unique(key, return_counts=True)
        keyu = np.unique(key)
        bk = (keyu >> 32).astype(np.int64)
        need = np.zeros(n_blocks * K, np.int64)
        np.maximum.at(need, bk, mult)
        S = np.maximum(S, need.reshape(n_blocks, K))

    def make_layout(S):
        sub2k = []
        sub_ids = []
        n_win = []
        for b in range(n_blocks):
            ks = []
            ids = [[] for _ in range(K)]
            for r in range(int(S[b].max())):
                for k in range(K):
                    if r < S[b, k]:
                        ids[k].append(len(ks))
                        ks.append(k)
            while len(ks) % SUB_PER_WIN:
                ks.append(0)
            sub2k.append(np.array(ks, np.int64))
            sub_ids.append(ids)
            n_win.append(len(ks) // SUB_PER_WIN)
        return sub2k, sub_ids, n_win

    class _PlacementFail(Exception):
        def __init__(self, b, k):
            self.b, self.k = b, k

    def place_all(sub2k, sub_ids, n_win):
        return [place_core(c, sub2k, sub_ids, n_win) for c in range(n_cores)]

    def place_core(c, sub2k, sub_ids, n_win):
        kc, bc, ic, oc = per_core_pairs[c]
        gidx_cols = []
        sidx_cols = []
        for b in range(n_blocks):
            nsub = len(sub2k[b])
            gmat = np.zeros((nsub, P), np.int64)
            smat = np.full((nsub, P), trash, np.int64)
            fill = np.zeros(nsub, np.int64)
            win_rows = [set() for _ in range(n_win[b])]
            mb = bc == b
            kb, ib, ob = kc[mb], ic[mb], oc[mb]
            order = np.lexsort((kb, ob))
            kb, ib, ob = kb[order], ib[order], ob[order]
            nS_of = np.array([len(sub_ids[b][k]) for k in range(K)])
            rot = np.zeros(K, np.int64)
            grp_start = np.concatenate(
                ([0], np.flatnonzero(ob[1:] != ob[:-1]) + 1, [len(ob)]))
            sub_rows = [dict() for _ in range(nsub)]

            def place(j, ro, k, allow_evict=True):
                ids = sub_ids[b][k]
                nS = len(ids)
                for probe in range(nS):
                    s = ids[int((rot[k] + probe) % nS)]
                    w = s // SUB_PER_WIN
                    if fill[s] < P and ro not in win_rows[w]:
                        gmat[s, fill[s]] = ib[j]
                        smat[s, fill[s]] = ro
                        sub_rows[s][ro] = int(fill[s])
                        fill[s] += 1
                        win_rows[w].add(ro)
                        rot[k] = (rot[k] + 1) % nS
                        return True
                if not allow_evict:
                    return False
                for probe in range(nS):
                    s = ids[int((rot[k] + probe) % nS)]
                    w = s // SUB_PER_WIN
                    if fill[s] >= P or ro not in win_rows[w]:
                        continue
                    for s2 in range(w * SUB_PER_WIN,
                                    min((w + 1) * SUB_PER_WIN, nsub)):
                        if ro in sub_rows[s2]:
                            slot2 = sub_rows[s2][ro]
                            k2 = int(sub2k[b][s2])
                            ri2 = int(gmat[s2, slot2])
                            win_rows[w].discard(ro)
                            last = fill[s2] - 1
                            mv_ro = int(smat[s2, last])
                            gmat[s2, slot2] = gmat[s2, last]
                            smat[s2, slot2] = smat[s2, last]
                            if mv_ro != trash and mv_ro in sub_rows[s2] \
                                    and sub_rows[s2][mv_ro] == last:
                                sub_rows[s2][mv_ro] = slot2
                            del sub_rows[s2][ro]
                            fill[s2] -= 1
                            ok2 = _place_raw(ri2, ro, k2, exclude_w=w)
                            if ok2:
                                gmat[s, fill[s]] = ib[j]
                                smat[s, fill[s]] = ro
                                sub_rows[s][ro] = int(fill[s])
                                fill[s] += 1
                                win_rows[w].add(ro)
                                return True
                            _undo_place(s2, ri2, ro, slot2, mv_ro)
                            win_rows[w].add(ro)
                            break
                return False

            def _place_raw(ri, ro, k, exclude_w=-1):
                ids = sub_ids[b][k]
                nS = len(ids)
                for probe in range(nS):
                    s = ids[int((rot[k] + probe) % nS)]
                    w = s // SUB_PER_WIN
                    if w == exclude_w:
                        continue
                    if fill[s] < P and ro not in win_rows[w]:
                        gmat[s, fill[s]] = ri
                        smat[s, fill[s]] = ro
                        sub_rows[s][ro] = int(fill[s])
                        fill[s] += 1
                        win_rows[w].add(ro)
                        rot[k] = (rot[k] + 1) % nS
                        return True
                return False

            def _undo_place(s2, ri, ro, slot2, mv_ro):
                last = fill[s2]
                if mv_ro != trash and mv_ro in sub_rows[s2] \
                        and sub_rows[s2][mv_ro] == slot2:
                    sub_rows[s2][mv_ro] = last
                gmat[s2, last] = gmat[s2, slot2]
                smat[s2, last] = smat[s2, slot2]
                gmat[s2, slot2] = ri
                smat[s2, slot2] = ro
                sub_rows[s2][ro] = slot2
                fill[s2] += 1

            sizes = np.diff(grp_start)
            for gi in np.argsort(-sizes, kind="stable"):
                lo, hi = grp_start[gi], grp_start[gi + 1]
                ro = int(ob[lo])
                occs = sorted(range(lo, hi), key=lambda j: nS_of[kb[j]])
                for j in occs:
                    ok = place(j, ro, int(kb[j]))
                    if not ok:
                        raise _PlacementFail(b, int(kb[j]))
            gidx_cols.append(gmat)
            sidx_cols.append(smat)
        gmat = np.concatenate(gidx_cols, 0)
        smat = np.concatenate(sidx_cols, 0)
        return (gmat, smat)

    per_core = None
    for _retry in range(64):
        sub2k, sub_ids, n_win = make_layout(S)
        try:
            per_core = place_all(sub2k, sub_ids, n_win)
            break
        except _PlacementFail as pf:
            S[pf.b, pf.k] += 1
    assert per_core is not None, "placement failed after retries"

    total_win = int(sum(n_win))
    sched = dict(rows_per_core=rpc, block_rows=block_rows, n_blocks=n_blocks,
                 sub2k=sub2k, n_win=n_win, total_win=total_win,
                 acc_rows=acc_rows, trash=trash,
                 feats_pad_rows=block_rows * n_blocks, K=K)

    # verify: within every window scatter rows (non-trash) distinct
    for c in range(n_cores):
        gmat, smat = per_core[c]
        off = 0
        for b in range(n_blocks):
            nsub = len(sub2k[b])
            sm = smat[off:off + nsub].reshape(-1, SUB_PER_WIN * P)
            for w in range(sm.shape[0]):
                rows = sm[w][sm[w] != trash]
                assert len(rows) == len(np.unique(rows)), (c, b, w)
            off += nsub
    return sched, per_core


def pack_idx16(mat):
    """[nsub, P] int -> wire layout [128, nwin * WIN_PAIRS//16] int16."""
    nsub = mat.shape[0]
    assert nsub % SUB_PER_WIN == 0
    nwin = nsub // SUB_PER_WIN
    cols = WIN_PAIRS // 16
    out = np.zeros((P, nwin * cols), np.int16)
    w = mat.reshape(nwin, WIN_PAIRS).astype(np.int16)
    for n in range(nwin):
        out[:, n * cols:(n + 1) * cols] = np.tile(w[n].reshape(-1, 16).T, (8, 1))
    return out


# ---------------------------------------------------------------------------
# device program
# ---------------------------------------------------------------------------

def build_program_raw(sched, c_in=256, c_out=128):
    K = sched["K"]
    n_blocks = sched["n_blocks"]
    block_rows = sched["block_rows"]
    acc_rows = sched["acc_rows"]
    rpc = sched["rows_per_core"]
    total_win = sched["total_win"]
    NG, NCS, NPS, NSW, DEPTH = 8, 8, 6, 2, 6
    NSEM_G = 8
    n_sweep = rpc // SWT
    IDXC = WIN_PAIRS // 16          # idx cols per window

    nc = bacc.Bacc("TRN2", target_bir_lowering=False, debug=False,
                   num_swdge_queues=NQ, dynamic_dma_scratch_size=32768)
    feats = nc.dram_tensor("feats", [sched["feats_pad_rows"], c_in],
                           mybir.dt.float16, kind="ExternalInput").ap()
    wmat = nc.dram_tensor("wmat", [P, K * 2 * c_out], mybir.dt.float16,
                          kind="ExternalInput").ap()
    bias_sw = nc.dram_tensor("bias_sw", [P, SWT // P, c_out], mybir.dt.float16,
                             kind="ExternalInput").ap()
    zero_sw = nc.dram_tensor("zero_sw", [P, SWT // P, c_out], mybir.dt.float16,
                             kind="ExternalInput").ap()
    gidx = nc.dram_tensor("gidx", [P, total_win * IDXC], mybir.dt.int16,
                          kind="ExternalInput").ap()
    sidx = nc.dram_tensor("sidx", [P, total_win * IDXC], mybir.dt.int16,
                          kind="ExternalInput").ap()
    accs = [nc.dram_tensor(f"acc{a}", [acc_rows, c_out], mybir.dt.float16,
                           kind="ExternalOutput").ap() for a in range(NACC)]
    out = nc.dram_tensor("out", [rpc, c_out], mybir.dt.float32,
                         kind="ExternalOutput").ap()

    sub_k = []
    win_block = []
    for b in range(n_blocks):
        win_block += [b] * sched["n_win"][b]
        sub_k.extend(int(x) for x in sched["sub2k"][b])
    U = len(sub_k)
    W = total_win
    assert U == SUB_PER_WIN * W

    with ExitStack() as stack:
        block = stack.enter_context(nc.Block())
        load_sem = stack.enter_context(nc.semaphore("load"))
        seed_sem = stack.enter_context(nc.semaphore("seed"))
        pe_sem = stack.enter_context(nc.semaphore("pe"))
        cp_sems = [stack.enter_context(nc.semaphore(f"cp{i}")) for i in range(2)]
        g_sems = [stack.enter_context(nc.semaphore(f"g{i}")) for i in range(NSEM_G)]
        s_sems = [stack.enter_context(nc.semaphore(f"s{i}")) for i in range(NQ)]
        swp_sems = [stack.enter_context(nc.semaphore(f"swp{i}")) for i in range(NSW)]
        swo_sems = [stack.enter_context(nc.semaphore(f"swo{i}")) for i in range(NSW)]
        dve_sw = stack.enter_context(nc.semaphore("dve_sw"))
        vtmp = stack.enter_context(nc.semaphore("vtmp"))
        act_sw = stack.enter_context(nc.semaphore("act_sw"))

        w_sb = stack.enter_context(
            nc.sbuf_tensor("w_sb", [P, K * 2 * c_out], mybir.dt.float16))
        bias_sb = stack.enter_context(
            nc.sbuf_tensor("bias_sb", [P, SWT // P, c_out], mybir.dt.float16))
        zero_sb = stack.enter_context(
            nc.sbuf_tensor("zero_sb", [P, SWT // P, c_out], mybir.dt.float16))
        gi_sb = stack.enter_context(
            nc.sbuf_tensor("gi_sb", [P, total_win * IDXC], mybir.dt.int16))
        si_sb = stack.enter_context(
            nc.sbuf_tensor("si_sb", [P, total_win * IDXC], mybir.dt.int16))
        g_sb = stack.enter_context(
            nc.sbuf_tensor("g_sb", [P, NG, 2, WIN_PAIRS], mybir.dt.float16))
        cs_sb = stack.enter_context(
            nc.sbuf_tensor("cs_sb", [P, NCS, SUB_PER_WIN, c_out],
                           mybir.dt.float16))
        sw_sb = stack.enter_context(
            nc.sbuf_tensor("sw_sb", [P, NSW, NACC, SWT // P, c_out],
                           mybir.dt.float16))
        r1_sb = stack.enter_context(
            nc.sbuf_tensor("r1_sb", [P, NSW, SWT // P, c_out], mybir.dt.float16))
        r2_sb = stack.enter_context(
            nc.sbuf_tensor("r2_sb", [P, NSW, SWT // P, c_out], mybir.dt.float16))
        r3_sb = stack.enter_context(
            nc.sbuf_tensor("r3_sb", [P, NSW, SWT // P, c_out], mybir.dt.float16))
        r_sb = stack.enter_context(
            nc.sbuf_tensor("r_sb", [P, NSW, SWT // P, c_out], mybir.dt.float32))
        cps = stack.enter_context(
            nc.psum_tensor("cps", [P, NPS, 512], mybir.dt.float32))

        LOAD_TOTAL = 16 * 5

        @block.sync
        def _(sy):
            sy.dma_start(out=w_sb[:], in_=wmat[:]).then_inc(load_sem, 16)
            sy.dma_start(out=bias_sb[:], in_=bias_sw[:]).then_inc(load_sem, 16)
            sy.dma_start(out=zero_sb[:], in_=zero_sw[:]).then_inc(load_sem, 16)
            sy.dma_start(out=gi_sb[:], in_=gidx[:]).then_inc(load_sem, 16)
            sy.dma_start(out=si_sb[:], in_=sidx[:]).then_inc(load_sem, 16)
            # seed acc0 tiles with bias, acc1 with zeros
            sy.wait_ge(load_sem, LOAD_TOTAL)
            for t in range(n_sweep):
                sy.dma_start(out=accs[0][t * SWT:(t + 1) * SWT, :],
                             in_=bias_sb[:]).then_inc(seed_sem, 16)
                for a in range(1, NACC):
                    sy.dma_start(out=accs[a][t * SWT:(t + 1) * SWT, :],
                                 in_=zero_sb[:]).then_inc(seed_sem, 16)
            # trash region seed (scatter-adds land there; keep deterministic)
            for a in range(NACC):
                sy.dma_start(out=accs[a][rpc:rpc + 256, :],
                             in_=zero_sb[:, 0:2, :]).then_inc(seed_sem, 16)
            # ---- final sweep ----
            for l in range(NQ):
                cnt = sum(1 for w in range(W) if w % NQ == l)
                sy.wait_ge(s_sems[l], 16 * cnt)
            LAG = 2
            for t in range(n_sweep + LAG):
                if t < n_sweep:
                    slot = t % NSW
                    if t >= NSW:
                        sy.wait_ge(dve_sw, t - NSW + 1)  # sw_sb slot consumed
                    for a in range(NACC):
                        sy.dma_start(out=sw_sb[:, slot, a, :, :],
                                     in_=accs[a][t * SWT:(t + 1) * SWT, :]
                                     ).then_inc(swp_sems[slot], 16)
                if t >= LAG:
                    st = t - LAG
                    sy.wait_ge(act_sw, st + 1)
                    sy.dma_start(out=out[st * SWT:(st + 1) * SWT, :],
                                 in_=r_sb[:, st % NSW, :, :]
                                 ).then_inc(swo_sems[st % NSW], 16)

        @block.tensor
        def _(pe):
            pe.wait_ge(load_sem, LOAD_TOTAL)
            for u in range(U):
                w = u // SUB_PER_WIN
                s = u % SUB_PER_WIN
                k = sub_k[u]
                if s == 0:
                    pe.wait_ge(g_sems[w % NSEM_G], 16 * (w // NSEM_G + 1))
                if u >= NPS:
                    up = u - NPS
                    pe.wait_ge(cp_sems[up % 2], up // 2 + 1)
                pe.matmul(out=cps[:, u % NPS, 0:c_out],
                          lhsT=g_sb[:, w % NG, 0, s * P:(s + 1) * P],
                          rhs=w_sb[:, (k * 2) * c_out:(k * 2 + 1) * c_out],
                          start=True, stop=False)
                pe.matmul(out=cps[:, u % NPS, 0:c_out],
                          lhsT=g_sb[:, w % NG, 1, s * P:(s + 1) * P],
                          rhs=w_sb[:, (k * 2 + 1) * c_out:(k * 2 + 2) * c_out],
                          start=False, stop=True).then_inc(pe_sem, 1)

        @block.scalar
        def _(sc):
            sc.wait_ge(load_sem, LOAD_TOTAL)
            for u in range(0, U, 2):                 # even u
                w = u // SUB_PER_WIN
                s = u % SUB_PER_WIN
                sc.wait_ge(pe_sem, u + 1)
                if s == 0 and w >= NCS:
                    lw = w - NCS
                    sc.wait_ge(s_sems[lw % NQ], 16 * (lw // NQ + 1))
                sc.copy(out=cs_sb[:, w % NCS, s, :],
                        in_=cps[:, u % NPS, 0:c_out]).then_inc(cp_sems[0], 1)
            for t in range(n_sweep):
                slot = t % NSW
                sc.wait_ge(dve_sw, t + 1)
                if t >= NSW:
                    sc.wait_ge(swo_sems[slot], 16 * (t // NSW))   # r_sb reuse
                sc.activation(out=r_sb[:, slot, :, :], in_=r3_sb[:, slot, :, :],
                              func=mybir.ActivationFunctionType.Relu
                              ).then_inc(act_sw, 1)

        @block.vector
        def _(ve):
            ve.wait_ge(load_sem, LOAD_TOTAL)
            for u in range(1, U, 2):                 # odd u
                w = u // SUB_PER_WIN
                s = u % SUB_PER_WIN
                ve.wait_ge(pe_sem, u + 1)
                if s == 1 and w >= NCS:
                    lw = w - NCS
                    ve.wait_ge(s_sems[lw % NQ], 16 * (lw // NQ + 1))
                ve.tensor_copy(out=cs_sb[:, w % NCS, s, :],
                               in_=cps[:, u % NPS, 0:c_out]).then_inc(cp_sems[1], 1)
            for t in range(n_sweep):
                slot = t % NSW
                ve.wait_ge(swp_sems[slot], 16 * NACC * (t // NSW + 1))
                if t >= NSW:
                    ve.wait_ge(act_sw, t - NSW + 1)   # r1 slot reuse
                ve.tensor_add(out=r1_sb[:, slot, :, :],
                              in0=sw_sb[:, slot, 0, :, :],
                              in1=sw_sb[:, slot, 1, :, :]).then_inc(vtmp, 1)
                ve.tensor_add(out=r2_sb[:, slot, :, :],
                              in0=sw_sb[:, slot, 2, :, :],
                              in1=sw_sb[:, slot, 3, :, :]).then_inc(vtmp, 1)
                ve.wait_ge(vtmp, 2 * (t + 1))
                ve.tensor_add(out=r3_sb[:, slot, :, :],
                              in0=r1_sb[:, slot, :, :],
                              in1=r2_sb[:, slot, :, :]).then_inc(dve_sw, 1)

        @block.gpsimd
        def _(gp):
            from concourse.library_config import mlp
            gp.load_library(mlp)
            gp.wait_ge(load_sem, LOAD_TOTAL)
            seed_total = 16 * (NACC * n_sweep + NACC)
            for w in range(W + DEPTH):
                if w < W:
                    b = win_block[w]
                    if w >= NG:
                        gp.wait_ge(pe_sem, SUB_PER_WIN * (w - NG + 1))  # g_sb reuse
                    gp.dma_gather(
                        g_sb[:, w % NG, :, :],
                        feats[b * block_rows:(b + 1) * block_rows, :],
                        gi_sb[:, w * IDXC:(w + 1) * IDXC],
                        WIN_PAIRS, WIN_PAIRS, c_in,
                        transpose=True, queue_num=w % NQ,
                    ).then_inc(g_sems[w % NSEM_G], 16)
                if w >= DEPTH:
                    ws = w - DEPTH
                    if ws == 0:
                        gp.wait_ge(seed_sem, seed_total)
                    gp.wait_ge(cp_sems[0], (SUB_PER_WIN // 2) * (ws + 1))
                    gp.wait_ge(cp_sems[1], (SUB_PER_WIN // 2) * (ws + 1))
                    if ws >= NACC:
                        pa = ws - NACC
                        gp.wait_ge(s_sems[pa % NQ], 16 * (pa // NQ + 1))
                    gp.dma_scatter_add(
                        accs[ws % NACC], cs_sb[:, ws % NCS, :, :],
                        si_sb[:, ws * IDXC:(ws + 1) * IDXC],
                        WIN_PAIRS, WIN_PAIRS, c_out,
                        queue_num=ws % NQ,
                    ).then_inc(s_sems[ws % NQ], 16)

    nc.compile()
    return nc


def make_inputs(feats, weight, bias, sched, per_core):
    
    n_in, c_in = feats.shape
    K, _, c_out = weight.shape
    fp = np.zeros((sched["feats_pad_rows"], c_in), np.float16)
    fp[:n_in] = feats.astype(np.float16)
    wm = np.ascontiguousarray(
        weight.astype(np.float32).reshape(K, 2, P, c_out).transpose(2, 0, 1, 3)
    ).reshape(P, K * 2 * c_out).astype(np.float16)
    bias_sw = np.tile(bias.astype(np.float16)[None, None, :], (P, SWT // P, 1))
    zero_sw = np.zeros((P, SWT // P, c_out), np.float16)
    in_maps = []
    for (gmat, smat) in per_core:
        in_maps.append(dict(feats=fp, wmat=wm, bias_sw=bias_sw, zero_sw=zero_sw,
                            gidx=pack_idx16(gmat), sidx=pack_idx16(smat)))
    return in_maps


# ---------------------------------------------------------------------------
# harness entry point
# ---------------------------------------------------------------------------

N_CORES = 8
_CACHE = {}


def kernel(feats, weight, bias, in_map, out_map, n_out):
    """Full-input entry: shards across 8 NeuronCores internally."""
    from concourse.bass_utils import run_bass_kernel_spmd

    feats = np.asarray(feats, dtype=np.float32)
    weight = np.asarray(weight, dtype=np.float32)
    bias = np.asarray(bias, dtype=np.float32)
    in_map = np.asarray(in_map)
    out_map = np.asarray(out_map)
    n_out = int(n_out)
    n_in = feats.shape[0]
    K = weight.shape[0]

    sched, per_core = build_schedule(in_map, out_map, n_in, n_out, N_CORES, K)
    in_maps = make_inputs(feats, weight, bias, sched, per_core)

    key = (n_in, n_out, K, sched["total_win"])
    nc = _CACHE.get(key)
    if nc is None:
        nc = build_program_raw(sched)
        _CACHE[key] = nc

    res = run_bass_kernel_spmd(nc, in_maps, list(range(N_CORES)))
    rpc = sched["rows_per_core"]
    got = np.concatenate([res.results[c]["out"][:rpc] for c in range(N_CORES)], 0)
    return np.ascontiguousarray(got[:n_out])
